# revision 3
# baseline (speedup 1.0000x reference)
"""Trainium2 Bass kernel for nn_CMSWrite (scatter_memory) — v3.

Changes vs baseline (531us):
1. AR pipelining: all three per-level softmax-normalizer AllReduce
   triggers are emitted BEFORE any AR-consuming op. In the baseline the
   inv/vq/kq scaling for level l sat in the in-order DVE queue between
   the level-l and level-(l+1) score passes, so each AR (12-42us for
   32 bytes!) serialized the next level's entire score pass; the three
   ARs completed at ~122/138/227us and starved the DMA for the first
   ~250us. Now triggers fire at ~37/58/80us back to back.
2. K resident: each level's K shard is loaded once (one 4.2MB DMA),
   used by the score pass, then updated in place (keep*K + ga*k) and
   written out. Phase 2 streams only M (in-place update in the stream
   tile). Traffic: 138.4 -> 125.8 MB/core = the read-once/write-once
   floor. No concat-out tile (saves 40KB/partition SBUF).
3. zcol via PE transpose instead of a DRAM round-trip (shortens the
   MLP critical path that gates each score pass / AR trigger).

Queue plan (all in-order per engine; no AR-dependent op may block a
prefetch stream):
  sync   : small rows, MLP weights, K level loads, mi (M chunk) loads
  scalar : ACT compute (LN/exp/tanh), ecr bounce writes, out writes
  gpsimd : memsets, cc_in writes, AR triggers, ei loads, zg loads

Slot permutation (per level, per core shard of S=8192 slots):
partition p, tile-column j in [0,64) holds slot p*64 + j; used
identically everywhere so it cancels. Per-partition DMA runs: K load
32KB contiguous, M chunk 16KB contiguous, out M chunk 8x2KB runs,
out K chunk 8x512B runs.
"""

import math
import numpy as np

L = 3
N = 65536
DLVL = 512
DK = 128
DZ = 128
NCORES = 8
S = N // NCORES          # 8192 slots per core
T = S // 128             # 64 slot-tiles of 128
SUB = 8                  # tile-columns per phase-2 chunk
NCH = T // SUB           # 8 chunks per level
EPS = 1e-5
THRESH = 0.1
SCALE = 1.0 / math.sqrt(DK)

_STATE = {}
SKIP_CC = False   # debug: replace AllReduce with local copy (wrong numerics)


def _build_bass():
    import concourse.bacc as bacc
    import concourse.tile as tile
    import concourse.mybir as mybir
    from concourse.masks import make_identity

    f32 = mybir.dt.float32
    bf16 = mybir.dt.bfloat16
    A = mybir.AluOpType
    AF = mybir.ActivationFunctionType
    AX = mybir.AxisListType

    nc = bacc.Bacc("TRN2", target_bir_lowering=False, debug=False,
                   num_devices=NCORES)

    Mp = nc.dram_tensor("Mp", [L, S, DLVL], f32, kind="ExternalInput").ap()
    Kp = nc.dram_tensor("Kp", [L, S, DK], f32, kind="ExternalInput").ap()
    xcatT = nc.dram_tensor("xcatT", [L, 128, 14], f32, kind="ExternalInput").ap()
    wevT = nc.dram_tensor("wevT", [L, 1792, 128], f32, kind="ExternalInput").ap()
    wvalT = nc.dram_tensor("wvalT", [L, 128, DLVL], f32, kind="ExternalInput").ap()
    wkeyT = nc.dram_tensor("wkeyT", [L, 128, DK], f32, kind="ExternalInput").ap()
    bev_r = nc.dram_tensor("bev_r", [1, L * DZ], f32, kind="ExternalInput").ap()
    lng_r = nc.dram_tensor("lng_r", [1, L * DZ], f32, kind="ExternalInput").ap()
    lnb_r = nc.dram_tensor("lnb_r", [1, L * DZ], f32, kind="ExternalInput").ap()
    wg_r = nc.dram_tensor("wg_r", [1, L * DZ], f32, kind="ExternalInput").ap()
    bg_r = nc.dram_tensor("bg_r", [1, L], f32, kind="ExternalInput").ap()
    bval_r = nc.dram_tensor("bval_r", [1, L * DLVL], f32, kind="ExternalInput").ap()
    bkey_r = nc.dram_tensor("bkey_r", [1, L * DK], f32, kind="ExternalInput").ap()
    dec_r = nc.dram_tensor("dec_r", [1, L], f32, kind="ExternalInput").ap()

    out = nc.dram_tensor("out", [L, S, DLVL + DK], f32, kind="ExternalOutput").ap()

    with tile.TileContext(nc) as tc:
        with (
            tc.tile_pool(name="constp", bufs=1) as constp,
            tc.tile_pool(name="wp", bufs=1) as wp,
            tc.tile_pool(name="sm", bufs=1) as sm,
            tc.tile_pool(name="zrp", bufs=3) as zrp,
            tc.tile_pool(name="junkp", bufs=3) as junkp,
            tc.tile_pool(name="kresp", bufs=1) as kresp,
            tc.tile_pool(name="mip", bufs=1) as mip,
            tc.tile_pool(name="eip", bufs=1) as eip,
            tc.tile_pool(name="pmisc", bufs=3, space="PSUM") as pmisc,
            tc.tile_pool(name="pmp", bufs=3, space="PSUM") as pmp,
            tc.tile_pool(name="pkp", bufs=2, space="PSUM") as pkp,
            tc.tile_pool(name="dramp", bufs=1, space="DRAM") as dramp,
        ):
            # ---------------- constants / small input rows ----------------
            ident = constp.tile([128, 128], f32, name="ident")
            make_identity(nc, ident[:])
            ones_row = constp.tile([1, 128], f32, name="ones_row")
            nc.gpsimd.memset(ones_row[:], 1.0)
            ones_col = constp.tile([128, 1], f32, name="ones_col")
            nc.gpsimd.memset(ones_col[:], 1.0)
            eps_sb = constp.tile([1, 1], f32, name="eps_sb")
            nc.gpsimd.memset(eps_sb[:], EPS)

            def _row(name, src, width):
                t = sm.tile([1, width], f32, name=name)
                nc.sync.dma_start(t[:], src)
                return t

            bev_sb = _row("bev_sb", bev_r[:], L * DZ)
            lng_sb = _row("lng_sb", lng_r[:], L * DZ)
            lnb_sb = _row("lnb_sb", lnb_r[:], L * DZ)
            wg_sb = _row("wg_sb", wg_r[:], L * DZ)
            bg_sb = _row("bg_sb", bg_r[:], L)
            bval_sb = _row("bval_sb", bval_r[:], L * DLVL)
            bkey_sb = _row("bkey_sb", bkey_r[:], L * DK)
            dec_sb = _row("dec_sb", dec_r[:], L)

            # keep = 1 - decay, broadcast to all 128 partitions
            keepr = sm.tile([1, L], f32, name="keepr")
            nc.scalar.activation(keepr[:], dec_sb[:], AF.Identity,
                                 bias=1.0, scale=-1.0)
            pkeep = pmisc.tile([128, L], f32, name="pkeep", tag="pmisc")
            nc.tensor.matmul(pkeep[:], lhsT=ones_row[:], rhs=keepr[:],
                             start=True, stop=True)
            keep_bc = sm.tile([128, L], f32, name="keep_bc")
            nc.vector.tensor_copy(keep_bc[:], pkeep[:])

            # persistent per-level results (all on partition 0 rows)
            vrow = sm.tile([1, L * DLVL], bf16, name="vrow")
            krow = sm.tile([1, L * DK], f32, name="krow")
            # bf16 copies feed the phase-2 rank-1 matmuls (4x PE rate);
            # the update term is ~1e-4 of the output so bf16 noise ~1e-7.
            vq = sm.tile([1, L * DLVL], bf16, name="vq")
            kq = sm.tile([1, L * DK], bf16, name="kq")
            geff = sm.tile([1, L], f32, name="geff")
            kbc = sm.tile([128, L * DK], f32, name="kbc")
            scores = sm.tile([128, L * T], f32, name="scores")
            zpart = sm.tile([128, L], f32, name="zpart")

            ecr = dramp.tile([L * S], bf16, name="ecr")
            cc_ins = [dramp.tile([1, 8], f32, name=f"cc_in{l}")
                      for l in range(L)]
            cc_outs = [dramp.tile([1, 8], f32, name=f"cc_out{l}",
                                  addr_space="Shared") for l in range(L)]

            # ---------------- phase 0+1: per-level MLP chains + score
            # passes, software-pipelined. Level 0's MLP runs first (its
            # AR trigger is the critical one); level l+1's chain is then
            # interleaved into level l's 64-op score pass in engine-run
            # groups, so its ~9 cross-engine hops (~2-3us each) hide
            # inside the score window instead of serializing after it.
            # NO AR-consuming op is emitted here, so the 3 ARs pipeline.
            zS = [slice(l * DZ, (l + 1) * DZ) for l in range(L)]
            kres = []

            def mlp_gen(l):
                zsl = zS[l]
                ksl = slice(l * DK, (l + 1) * DK)
                vsl = slice(l * DLVL, (l + 1) * DLVL)
                # group 0: input loads (incl. this level's resident K) and
                # the z-preactivation PSUM chain on the PE.
                xc = wp.tile([128, 14], f32, name="xc", bufs=2)
                nc.sync.dma_start(xc[:], xcatT[l])
                wev = wp.tile([128, 14, 128], f32, name="wev", bufs=1)
                nc.sync.dma_start(
                    wev[:], wevT[l].rearrange("(c p) j -> p c j", p=128))
                kr = kresp.tile([128, T, DK], f32, name=f"kres{l}")
                kres.append(kr)
                nc.sync.dma_start(
                    kr[:], Kp[l].rearrange("(p t) d -> p t d", t=T))
                pz = pmisc.tile([1, 128], f32, name="pz", tag="pmisc")
                for c in range(14):
                    nc.tensor.matmul(pz[:], lhsT=xc[:, c:c + 1],
                                     rhs=wev[:, c, :],
                                     start=(c == 0), stop=(c == 13))
                yield
                # V-run: bias add + LN stats
                zr = zrp.tile([1, 128], f32, name="zr", bufs=2)
                nc.vector.tensor_tensor(zr[:], pz[:], bev_sb[:, zsl], op=A.add)
                musum = zrp.tile([1, 1], f32, name="musum", bufs=2)
                nc.vector.tensor_reduce(musum[:], zr[:], axis=AX.X, op=A.add)
                mu = zrp.tile([1, 1], f32, name="mu", bufs=2)
                nc.vector.tensor_scalar(mu[:], musum[:], 1.0 / DZ, None,
                                        A.mult)
                zm = zrp.tile([1, 128], f32, name="zm", bufs=2)
                nc.vector.tensor_scalar(zm[:], zr[:], mu[:], None, A.subtract)
                jr = junkp.tile([1, 128], f32, name="jr", bufs=2)
                nc.vector.tensor_tensor(jr[:], zm[:], zm[:], op=A.mult)
                vsum = zrp.tile([1, 1], f32, name="vsum", bufs=2)
                nc.vector.tensor_reduce(vsum[:], jr[:], axis=AX.X, op=A.add)
                yield
                # ACT hop: std; also drop the wval/wkey loads here so they
                # sit late in the sync queue (behind the K loads).
                wval = wp.tile([128, DLVL], f32, name="wval", bufs=2)
                nc.sync.dma_start(wval[:], wvalT[l])
                wkey = wp.tile([128, DK], f32, name="wkey", bufs=2)
                nc.sync.dma_start(wkey[:], wkeyT[l])
                std = zrp.tile([1, 1], f32, name="std", bufs=2)
                nc.scalar.activation(std[:], vsum[:], AF.Sqrt, bias=eps_sb[:],
                                     scale=1.0 / DZ)
                yield
                # V-run: normalize + relu + gate dot
                rstd = zrp.tile([1, 1], f32, name="rstd", bufs=2)
                nc.vector.reciprocal(rstd[:], std[:])
                zs2 = zrp.tile([1, 128], f32, name="zs2", bufs=2)
                nc.vector.scalar_tensor_tensor(
                    out=zs2[:], in0=zm[:], scalar=rstd[:],
                    in1=lng_sb[:, zsl], op0=A.mult, op1=A.mult)
                zs3 = zrp.tile([1, 128], f32, name="zs3", bufs=2)
                nc.vector.tensor_tensor(zs3[:], zs2[:], lnb_sb[:, zsl],
                                        op=A.add)
                zrow = zrp.tile([1, 128], f32, name="zrow", bufs=2)
                nc.vector.tensor_scalar(zrow[:], zs3[:], 0.0, None, A.max)
                jg = junkp.tile([1, 128], f32, name="jg", bufs=2)
                nc.vector.tensor_tensor(jg[:], zrow[:], wg_sb[:, zsl],
                                        op=A.mult)
                gd = zrp.tile([1, 1], f32, name="gd", bufs=2)
                nc.vector.tensor_reduce(gd[:], jg[:], axis=AX.X, op=A.add)
                yield
                # ACT hop: gate sigmoid; PE: z transpose
                gsig = zrp.tile([1, 1], f32, name="gsig", bufs=2)
                nc.scalar.activation(gsig[:], gd[:], AF.Sigmoid,
                                     bias=bg_sb[:, l:l + 1], scale=1.0)
                pzc = pmisc.tile([128, 1], f32, name="pzc", tag="pmisc")
                nc.tensor.transpose(pzc[:], zrow[:], ident[0:1, 0:1])
                yield
                # V-run: gate threshold + zcol copy
                msk = zrp.tile([1, 1], f32, name="msk", bufs=2)
                nc.vector.tensor_scalar(msk[:], gsig[:], THRESH, None,
                                        A.is_ge)
                nc.vector.tensor_tensor(geff[:, l:l + 1], gsig[:], msk[:],
                                        op=A.mult)
                zcol = zrp.tile([128, 1], f32, name="zcol", bufs=2)
                nc.vector.tensor_copy(zcol[:], pzc[:])
                yield
                # PE: value/key matvecs
                pv = pmisc.tile([1, DLVL], f32, name="pv", tag="pmisc")
                nc.tensor.matmul(pv[:], lhsT=zcol[:], rhs=wval[:],
                                 start=True, stop=True)
                pk0 = pmisc.tile([1, DK], f32, name="pk0", tag="pmisc")
                nc.tensor.matmul(pk0[:], lhsT=zcol[:], rhs=wkey[:],
                                 start=True, stop=True)
                yield
                # V-run: krow / ksc / vpre
                nc.vector.tensor_tensor(krow[:, ksl], pk0[:], bkey_sb[:, ksl],
                                        op=A.add)
                ksc = zrp.tile([1, DK], f32, name="ksc", bufs=2)
                nc.vector.tensor_scalar(ksc[:], krow[:, ksl], SCALE, None,
                                        A.mult)
                vpre = zrp.tile([1, DLVL], f32, name="vpre", bufs=2)
                nc.vector.tensor_tensor(vpre[:], pv[:], bval_sb[:, vsl],
                                        op=A.add)
                yield
                # PE: k broadcast; ACT: tanh (off the score critical path)
                pkb = pmisc.tile([128, DK], f32, name="pkb", tag="pmisc")
                nc.tensor.matmul(pkb[:], lhsT=ones_row[:], rhs=ksc[:],
                                 start=True, stop=True)
                nc.scalar.activation(vrow[:, vsl], vpre[:], AF.Tanh)
                yield
                # V: kbc broadcast copy -> score pass for this level is go
                nc.vector.tensor_copy(kbc[:, ksl], pkb[:])

            for _ in mlp_gen(0):
                pass
            for l in range(L):
                ksl = slice(l * DK, (l + 1) * DK)
                kr = kres[l]
                nxt = iter(mlp_gen(l + 1)) if l + 1 < L else None
                if nxt is not None:
                    next(nxt, None)
                for j in range(T):
                    lt = l * T + j
                    jk = junkp.tile([128, 128], bf16, name="jk", bufs=2)
                    nc.vector.scalar_tensor_tensor(
                        out=jk[:], in0=kr[:, j, :], scalar=1.0,
                        in1=kbc[:, ksl], op0=A.mult, op1=A.mult,
                        accum_out=scores[:, lt:lt + 1])
                    if nxt is not None and j % 6 == 5:
                        next(nxt, None)
                if nxt is not None:
                    for _ in nxt:
                        pass
                nc.scalar.activation(scores[:, l * T:(l + 1) * T],
                                     scores[:, l * T:(l + 1) * T], AF.Exp)
                nc.vector.tensor_reduce(zpart[:, l:l + 1],
                                        scores[:, l * T:(l + 1) * T],
                                        axis=AX.X, op=A.add)

                pt = pmisc.tile([64, 128], f32, name="pt", tag="pmisc")
                nc.tensor.transpose(pt[:], scores[:, l * T:(l + 1) * T],
                                    ident[:])
                et = zrp.tile([64, 128], bf16, name="et", bufs=1)
                nc.vector.tensor_copy(et[:], pt[:])
                nc.scalar.dma_start(
                    ecr[l * S:(l + 1) * S].rearrange("(t s) -> t s", s=128),
                    et[:])

                pz1 = pmisc.tile([1, 1], f32, name="pz1", tag="pmisc")
                nc.tensor.matmul(pz1[:], lhsT=ones_col[:],
                                 rhs=zpart[:, l:l + 1], start=True, stop=True)
                z1 = zrp.tile([1, 8], f32, name="z1")
                nc.gpsimd.memset(z1[:], 0.0)
                nc.vector.tensor_copy(z1[:, 0:1], pz1[:])

                nc.gpsimd.dma_start(cc_ins[l][:], z1[:])
                if not SKIP_CC:
                    nc.gpsimd.collective_compute(
                        "AllReduce", A.add,
                        replica_groups=[list(range(NCORES))],
                        ins=[cc_ins[l].opt()], outs=[cc_outs[l].opt()])
                else:
                    nc.gpsimd.dma_start(cc_outs[l][:], cc_ins[l][:])

            # ---- AR consumers + phase 2, interleaved per level so that no
            # AR wait sits ahead of another level's work in any in-order
            # engine queue. Per level: prefetch half the ei (exp-row)
            # tiles, then zg (the AR wait), then inv/vq/kq, remaining ei,
            # then the streaming loop.
            inv = sm.tile([1, L], f32, name="inv")

            # ei/zg loads go on the scalar (HWDGE) queue, NOT gpsimd: the
            # collective_compute triggers occupy the gpsimd queue until
            # their AR completes, so anything queued behind them would
            # inherit the full AR latency.
            def ei_load(l, c):
                ei = eip.tile([1, SUB * 128], bf16, name="ei", bufs=4)
                nc.scalar.dma_start(
                    ei[:],
                    ecr[l * S + c * SUB * 128:
                        l * S + (c + 1) * SUB * 128].rearrange(
                        "(a x) -> a x", a=1))
                return ei

            # AR-consumer block for level l: ei prefetch, zg (the AR wait),
            # then inv/vq/kq. Consumer(0) runs before the first chunk loop;
            # consumer(l+1) is hoisted into the MIDDLE of level l's chunk
            # loop (after chunk 6's compute) so that by the time level l+1
            # streaming starts, vq/kq are already computed and the level
            # transition costs ~2 engine hops instead of ~8 (~25us each).
            eis_all = {}

            def ar_consumer(l):
                vsl = slice(l * DLVL, (l + 1) * DLVL)
                ksl = slice(l * DK, (l + 1) * DK)
                eis_all[l] = {c: ei_load(l, c) for c in range(3)}
                zg = zrp.tile([1, 8], f32, name="zg")
                nc.scalar.dma_start(zg[:], cc_outs[l][:])
                zrcp = zrp.tile([1, 1], f32, name="zrcp")
                nc.vector.reciprocal(zrcp[:], zg[:, 0:1])
                nc.vector.tensor_tensor(inv[:, l:l + 1], geff[:, l:l + 1],
                                        zrcp[:], op=A.mult)
                nc.vector.tensor_scalar(vq[:, vsl], vrow[:, vsl],
                                        inv[:, l:l + 1], None, A.mult)
                nc.vector.tensor_scalar(kq[:, ksl], krow[:, ksl],
                                        inv[:, l:l + 1], None, A.mult)

            ar_consumer(0)
            for l in range(L):
                vsl = slice(l * DLVL, (l + 1) * DLVL)
                ksl = slice(l * DK, (l + 1) * DK)
                eis = eis_all[l]
                keep_sc = keep_bc[:, l:l + 1]
                kr = kres[l]
                outv = out[l].rearrange("(p t) d -> p t d", t=T)
                for c in range(NCH):
                    cs = slice(c * SUB, (c + 1) * SUB)
                    ei = eis.pop(c)
                    if c + 3 < NCH:
                        eis[c + 3] = ei_load(l, c + 3)
                    mi = mip.tile([128, SUB, DLVL], f32, name="mi", bufs=3)
                    nc.sync.dma_start(
                        mi[:],
                        Mp[l].rearrange("(p t) d -> p t d", t=T)[:, cs, :])
                    for t in range(SUB):
                        et_sl = ei[:, t * 128:(t + 1) * 128]
                        j = c * SUB + t
                        pm = pmp.tile([128, DLVL], f32, name="pm", tag="pm")
                        nc.tensor.matmul(pm[:], lhsT=et_sl,
                                         rhs=vq[:, vsl], start=True, stop=True)
                        nc.vector.scalar_tensor_tensor(
                            out=mi[:, t, :], in0=mi[:, t, :],
                            scalar=keep_sc, in1=pm[:], op0=A.mult, op1=A.add)
                        pkk = pkp.tile([128, DK], f32, name="pkk", tag="pk")
                        nc.tensor.matmul(pkk[:], lhsT=et_sl,
                                         rhs=kq[:, ksl], start=True, stop=True)
                        nc.vector.scalar_tensor_tensor(
                            out=kr[:, j, :], in0=kr[:, j, :],
                            scalar=keep_sc, in1=pkk[:], op0=A.mult, op1=A.add)
                    nc.scalar.dma_start(outv[:, cs, 0:DLVL], mi[:])
                    nc.scalar.dma_start(outv[:, cs, DLVL:DLVL + DK],
                                        kr[:, cs, :])
                    if c == 6 and l + 1 < L:
                        ar_consumer(l + 1)

    nc.compile()
    return nc


def _prep_in_maps(inputs):
    f32 = np.float32
    s_t = np.asarray(inputs["s_t"], f32)
    e_t = np.asarray(inputs["e_t"], f32)
    lc = np.asarray(inputs["level_contexts"], f32)
    W_ev0 = np.asarray(inputs["W_ev0"], f32)
    W_ev = np.asarray(inputs["W_ev"], f32)
    b_ev = np.asarray(inputs["b_ev"], f32)
    ln_g = np.asarray(inputs["ln_g"], f32)
    ln_b = np.asarray(inputs["ln_b"], f32)
    W_gate = np.asarray(inputs["W_gate"], f32)
    b_gate = np.asarray(inputs["b_gate"], f32)
    W_val = np.asarray(inputs["W_val"], f32)
    b_val = np.asarray(inputs["b_val"], f32)
    W_key = np.asarray(inputs["W_key"], f32)
    b_key = np.asarray(inputs["b_key"], f32)
    M = np.asarray(inputs["M"], f32)
    K = np.asarray(inputs["K"], f32)
    decay = np.asarray(inputs["decay"], f32)

    # unified MLP input per level: level 0 uses [s, e, 0-pad], levels 1-2 use
    # [s, ctx, e]; weight matrices padded/stacked to match.
    xcat = np.zeros((L, 1792), f32)
    xcat[0, :1024] = s_t
    xcat[0, 1024:1536] = e_t
    for l in (1, 2):
        xcat[l] = np.concatenate([s_t, lc[l - 1], e_t])
    xcatT = np.ascontiguousarray(
        xcat.reshape(L, 14, 128).transpose(0, 2, 1))
    W0p = np.concatenate([W_ev0, np.zeros((DZ, 256), f32)], axis=1)
    Wfull = np.stack([W0p, W_ev[0], W_ev[1]])
    wevT = np.ascontiguousarray(Wfull.transpose(0, 2, 1))
    wvalT = np.ascontiguousarray(W_val.transpose(0, 2, 1))
    wkeyT = np.ascontiguousarray(W_key.transpose(0, 2, 1))

    shared = dict(
        xcatT=xcatT, wevT=wevT, wvalT=wvalT, wkeyT=wkeyT,
        bev_r=b_ev.reshape(1, -1), lng_r=ln_g.reshape(1, -1),
        lnb_r=ln_b.reshape(1, -1), wg_r=W_gate.reshape(1, -1),
        bg_r=b_gate.reshape(1, -1), bval_r=b_val.reshape(1, -1),
        bkey_r=b_key.reshape(1, -1), dec_r=decay.reshape(1, -1),
    )
    in_maps = []
    for c in range(NCORES):
        sl = slice(c * S, (c + 1) * S)
        m = dict(shared)
        m["Mp"] = np.ascontiguousarray(M[:, sl, :])
        m["Kp"] = np.ascontiguousarray(K[:, sl, :])
        in_maps.append(m)
    return in_maps


def _run(inputs, trace=False):
    import concourse.bass_utils as bass_utils

    nc = _STATE.get("nc")
    if nc is None:
        nc = _build_bass()
        _STATE["nc"] = nc
    in_maps = _prep_in_maps(inputs)
    res = bass_utils.run_bass_kernel_spmd(
        nc, in_maps, core_ids=list(range(NCORES)), trace=trace)
    full = np.concatenate([res.results[c]["out"] for c in range(NCORES)],
                          axis=1)
    return full.astype(np.float32, copy=False), res


def kernel(**inputs):
    out, _ = _run(inputs, trace=False)
    return out


# revision 4
# speedup vs baseline: 1.0092x; 1.0092x over previous
"""Trainium2 Bass kernel for nn_CMSWrite (scatter_memory) — v3.

Changes vs baseline (531us):
1. AR pipelining: all three per-level softmax-normalizer AllReduce
   triggers are emitted BEFORE any AR-consuming op. In the baseline the
   inv/vq/kq scaling for level l sat in the in-order DVE queue between
   the level-l and level-(l+1) score passes, so each AR (12-42us for
   32 bytes!) serialized the next level's entire score pass; the three
   ARs completed at ~122/138/227us and starved the DMA for the first
   ~250us. Now triggers fire at ~37/58/80us back to back.
2. K resident: each level's K shard is loaded once (one 4.2MB DMA),
   used by the score pass, then updated in place (keep*K + ga*k) and
   written out. Phase 2 streams only M (in-place update in the stream
   tile). Traffic: 138.4 -> 125.8 MB/core = the read-once/write-once
   floor. No concat-out tile (saves 40KB/partition SBUF).
3. zcol via PE transpose instead of a DRAM round-trip (shortens the
   MLP critical path that gates each score pass / AR trigger).

Queue plan (all in-order per engine; no AR-dependent op may block a
prefetch stream):
  sync   : small rows, MLP weights, K level loads, mi (M chunk) loads
  scalar : ACT compute (LN/exp/tanh), ecr bounce writes, out writes
  gpsimd : memsets, cc_in writes, AR triggers, ei loads, zg loads

Slot permutation (per level, per core shard of S=8192 slots):
partition p, tile-column j in [0,64) holds slot p*64 + j; used
identically everywhere so it cancels. Per-partition DMA runs: K load
32KB contiguous, M chunk 16KB contiguous, out M chunk 8x2KB runs,
out K chunk 8x512B runs.
"""

import math
import numpy as np

L = 3
N = 65536
DLVL = 512
DK = 128
DZ = 128
NCORES = 8
S = N // NCORES          # 8192 slots per core
T = S // 128             # 64 slot-tiles of 128
SUB = 8                  # tile-columns per phase-2 chunk
NCH = T // SUB           # 8 chunks per level
EPS = 1e-5
THRESH = 0.1
SCALE = 1.0 / math.sqrt(DK)

_STATE = {}
SKIP_CC = False   # debug: replace AllReduce with local copy (wrong numerics)


def _build_bass():
    import concourse.bacc as bacc
    import concourse.tile as tile
    import concourse.mybir as mybir
    from concourse.masks import make_identity

    f32 = mybir.dt.float32
    bf16 = mybir.dt.bfloat16
    A = mybir.AluOpType
    AF = mybir.ActivationFunctionType
    AX = mybir.AxisListType

    nc = bacc.Bacc("TRN2", target_bir_lowering=False, debug=False,
                   num_devices=NCORES)

    Mp = nc.dram_tensor("Mp", [L, S, DLVL], f32, kind="ExternalInput").ap()
    Kp = nc.dram_tensor("Kp", [L, S, DK], f32, kind="ExternalInput").ap()
    xcatT = nc.dram_tensor("xcatT", [L, 128, 14], f32, kind="ExternalInput").ap()
    wevT = nc.dram_tensor("wevT", [L, 1792, 128], f32, kind="ExternalInput").ap()
    wvalT = nc.dram_tensor("wvalT", [L, 128, DLVL], f32, kind="ExternalInput").ap()
    wkeyT = nc.dram_tensor("wkeyT", [L, 128, DK], f32, kind="ExternalInput").ap()
    bev_r = nc.dram_tensor("bev_r", [1, L * DZ], f32, kind="ExternalInput").ap()
    lng_r = nc.dram_tensor("lng_r", [1, L * DZ], f32, kind="ExternalInput").ap()
    lnb_r = nc.dram_tensor("lnb_r", [1, L * DZ], f32, kind="ExternalInput").ap()
    wg_r = nc.dram_tensor("wg_r", [1, L * DZ], f32, kind="ExternalInput").ap()
    bg_r = nc.dram_tensor("bg_r", [1, L], f32, kind="ExternalInput").ap()
    bval_r = nc.dram_tensor("bval_r", [1, L * DLVL], f32, kind="ExternalInput").ap()
    bkey_r = nc.dram_tensor("bkey_r", [1, L * DK], f32, kind="ExternalInput").ap()
    dec_r = nc.dram_tensor("dec_r", [1, L], f32, kind="ExternalInput").ap()

    out = nc.dram_tensor("out", [L, S, DLVL + DK], f32, kind="ExternalOutput").ap()

    with tile.TileContext(nc) as tc:
        with (
            tc.tile_pool(name="constp", bufs=1) as constp,
            tc.tile_pool(name="wp", bufs=1) as wp,
            tc.tile_pool(name="sm", bufs=1) as sm,
            tc.tile_pool(name="zrp", bufs=3) as zrp,
            tc.tile_pool(name="junkp", bufs=3) as junkp,
            tc.tile_pool(name="kresp", bufs=1) as kresp,
            tc.tile_pool(name="ktp", bufs=1) as ktp,
            tc.tile_pool(name="mip", bufs=1) as mip,
            tc.tile_pool(name="eip", bufs=1) as eip,
            tc.tile_pool(name="pmisc", bufs=3, space="PSUM") as pmisc,
            tc.tile_pool(name="pmp", bufs=3, space="PSUM") as pmp,
            tc.tile_pool(name="pkp", bufs=2, space="PSUM") as pkp,
            tc.tile_pool(name="dramp", bufs=1, space="DRAM") as dramp,
        ):
            # ---------------- constants / small input rows ----------------
            ident = constp.tile([128, 128], f32, name="ident")
            make_identity(nc, ident[:])
            ones_row = constp.tile([1, 128], f32, name="ones_row")
            nc.gpsimd.memset(ones_row[:], 1.0)
            ones_col = constp.tile([128, 1], f32, name="ones_col")
            nc.gpsimd.memset(ones_col[:], 1.0)
            eps_sb = constp.tile([1, 1], f32, name="eps_sb")
            nc.gpsimd.memset(eps_sb[:], EPS)

            def _row(name, src, width):
                t = sm.tile([1, width], f32, name=name)
                nc.sync.dma_start(t[:], src)
                return t

            bev_sb = _row("bev_sb", bev_r[:], L * DZ)
            lng_sb = _row("lng_sb", lng_r[:], L * DZ)
            lnb_sb = _row("lnb_sb", lnb_r[:], L * DZ)
            wg_sb = _row("wg_sb", wg_r[:], L * DZ)
            bg_sb = _row("bg_sb", bg_r[:], L)
            bval_sb = _row("bval_sb", bval_r[:], L * DLVL)
            bkey_sb = _row("bkey_sb", bkey_r[:], L * DK)
            dec_sb = _row("dec_sb", dec_r[:], L)

            # keep = 1 - decay, broadcast to all 128 partitions
            keepr = sm.tile([1, L], f32, name="keepr")
            nc.scalar.activation(keepr[:], dec_sb[:], AF.Identity,
                                 bias=1.0, scale=-1.0)
            pkeep = pmisc.tile([128, L], f32, name="pkeep", tag="pmisc")
            nc.tensor.matmul(pkeep[:], lhsT=ones_row[:], rhs=keepr[:],
                             start=True, stop=True)
            keep_bc = sm.tile([128, L], f32, name="keep_bc")
            nc.vector.tensor_copy(keep_bc[:], pkeep[:])

            # persistent per-level results (all on partition 0 rows)
            vrow = sm.tile([1, L * DLVL], bf16, name="vrow")
            krow = sm.tile([1, L * DK], f32, name="krow")
            # bf16 copies feed the phase-2 rank-1 matmuls (4x PE rate);
            # the update term is ~1e-4 of the output so bf16 noise ~1e-7.
            vq = sm.tile([1, L * DLVL], bf16, name="vq")
            kq = sm.tile([1, L * DK], bf16, name="kq")
            geff = sm.tile([1, L], f32, name="geff")
            kbc = sm.tile([128, L * DK], f32, name="kbc")
            scores = sm.tile([128, L * T], f32, name="scores")
            zpart = sm.tile([128, L], f32, name="zpart")

            ecr = dramp.tile([L * S], bf16, name="ecr")
            cc_ins = [dramp.tile([1, 8], f32, name=f"cc_in{l}")
                      for l in range(L)]
            cc_outs = [dramp.tile([1, 8], f32, name=f"cc_out{l}",
                                  addr_space="Shared") for l in range(L)]

            # ---------------- phase 0+1: per-level MLP chains + score
            # passes, software-pipelined. Level 0's MLP runs first (its
            # AR trigger is the critical one); level l+1's chain is then
            # interleaved into level l's 64-op score pass in engine-run
            # groups, so its ~9 cross-engine hops (~2-3us each) hide
            # inside the score window instead of serializing after it.
            # NO AR-consuming op is emitted here, so the 3 ARs pipeline.
            zS = [slice(l * DZ, (l + 1) * DZ) for l in range(L)]
            kres = []

            def mlp_gen(l):
                zsl = zS[l]
                ksl = slice(l * DK, (l + 1) * DK)
                vsl = slice(l * DLVL, (l + 1) * DLVL)
                # group 0: input loads (incl. this level's resident K) and
                # the z-preactivation PSUM chain on the PE.
                xc = wp.tile([128, 14], f32, name="xc", bufs=2)
                nc.sync.dma_start(xc[:], xcatT[l])
                wev = wp.tile([128, 14, 128], f32, name="wev", bufs=1)
                nc.sync.dma_start(
                    wev[:], wevT[l].rearrange("(c p) j -> p c j", p=128))
                kr = kresp.tile([128, T, DK], bf16, name=f"kres{l}")
                kres.append(kr)
                pz = pmisc.tile([1, 128], f32, name="pz", tag="pmisc")
                for c in range(14):
                    nc.tensor.matmul(pz[:], lhsT=xc[:, c:c + 1],
                                     rhs=wev[:, c, :],
                                     start=(c == 0), stop=(c == 13))
                yield
                # V-run: bias add + LN stats
                zr = zrp.tile([1, 128], f32, name="zr", bufs=2)
                nc.vector.tensor_tensor(zr[:], pz[:], bev_sb[:, zsl], op=A.add)
                musum = zrp.tile([1, 1], f32, name="musum", bufs=2)
                nc.vector.tensor_reduce(musum[:], zr[:], axis=AX.X, op=A.add)
                mu = zrp.tile([1, 1], f32, name="mu", bufs=2)
                nc.vector.tensor_scalar(mu[:], musum[:], 1.0 / DZ, None,
                                        A.mult)
                zm = zrp.tile([1, 128], f32, name="zm", bufs=2)
                nc.vector.tensor_scalar(zm[:], zr[:], mu[:], None, A.subtract)
                jr = junkp.tile([1, 128], f32, name="jr", bufs=2)
                nc.vector.tensor_tensor(jr[:], zm[:], zm[:], op=A.mult)
                vsum = zrp.tile([1, 1], f32, name="vsum", bufs=2)
                nc.vector.tensor_reduce(vsum[:], jr[:], axis=AX.X, op=A.add)
                yield
                # ACT hop: std; also drop the wval/wkey loads here so they
                # sit late in the sync queue (behind the K loads).
                wval = wp.tile([128, DLVL], f32, name="wval", bufs=2)
                nc.sync.dma_start(wval[:], wvalT[l])
                wkey = wp.tile([128, DK], f32, name="wkey", bufs=2)
                nc.sync.dma_start(wkey[:], wkeyT[l])
                std = zrp.tile([1, 1], f32, name="std", bufs=2)
                nc.scalar.activation(std[:], vsum[:], AF.Sqrt, bias=eps_sb[:],
                                     scale=1.0 / DZ)
                yield
                # V-run: normalize + relu + gate dot
                rstd = zrp.tile([1, 1], f32, name="rstd", bufs=2)
                nc.vector.reciprocal(rstd[:], std[:])
                zs2 = zrp.tile([1, 128], f32, name="zs2", bufs=2)
                nc.vector.scalar_tensor_tensor(
                    out=zs2[:], in0=zm[:], scalar=rstd[:],
                    in1=lng_sb[:, zsl], op0=A.mult, op1=A.mult)
                zs3 = zrp.tile([1, 128], f32, name="zs3", bufs=2)
                nc.vector.tensor_tensor(zs3[:], zs2[:], lnb_sb[:, zsl],
                                        op=A.add)
                zrow = zrp.tile([1, 128], f32, name="zrow", bufs=2)
                nc.vector.tensor_scalar(zrow[:], zs3[:], 0.0, None, A.max)
                jg = junkp.tile([1, 128], f32, name="jg", bufs=2)
                nc.vector.tensor_tensor(jg[:], zrow[:], wg_sb[:, zsl],
                                        op=A.mult)
                gd = zrp.tile([1, 1], f32, name="gd", bufs=2)
                nc.vector.tensor_reduce(gd[:], jg[:], axis=AX.X, op=A.add)
                yield
                # ACT hop: gate sigmoid; PE: z transpose
                gsig = zrp.tile([1, 1], f32, name="gsig", bufs=2)
                nc.scalar.activation(gsig[:], gd[:], AF.Sigmoid,
                                     bias=bg_sb[:, l:l + 1], scale=1.0)
                pzc = pmisc.tile([128, 1], f32, name="pzc", tag="pmisc")
                nc.tensor.transpose(pzc[:], zrow[:], ident[0:1, 0:1])
                yield
                # V-run: gate threshold + zcol copy
                msk = zrp.tile([1, 1], f32, name="msk", bufs=2)
                nc.vector.tensor_scalar(msk[:], gsig[:], THRESH, None,
                                        A.is_ge)
                nc.vector.tensor_tensor(geff[:, l:l + 1], gsig[:], msk[:],
                                        op=A.mult)
                zcol = zrp.tile([128, 1], f32, name="zcol", bufs=2)
                nc.vector.tensor_copy(zcol[:], pzc[:])
                yield
                # PE: value/key matvecs
                pv = pmisc.tile([1, DLVL], f32, name="pv", tag="pmisc")
                nc.tensor.matmul(pv[:], lhsT=zcol[:], rhs=wval[:],
                                 start=True, stop=True)
                pk0 = pmisc.tile([1, DK], f32, name="pk0", tag="pmisc")
                nc.tensor.matmul(pk0[:], lhsT=zcol[:], rhs=wkey[:],
                                 start=True, stop=True)
                yield
                # V-run: krow / ksc / vpre
                nc.vector.tensor_tensor(krow[:, ksl], pk0[:], bkey_sb[:, ksl],
                                        op=A.add)
                ksc = zrp.tile([1, DK], f32, name="ksc", bufs=2)
                nc.vector.tensor_scalar(ksc[:], krow[:, ksl], SCALE, None,
                                        A.mult)
                vpre = zrp.tile([1, DLVL], f32, name="vpre", bufs=2)
                nc.vector.tensor_tensor(vpre[:], pv[:], bval_sb[:, vsl],
                                        op=A.add)
                yield
                # PE: k broadcast; ACT: tanh (off the score critical path)
                pkb = pmisc.tile([128, DK], f32, name="pkb", tag="pmisc")
                nc.tensor.matmul(pkb[:], lhsT=ones_row[:], rhs=ksc[:],
                                 start=True, stop=True)
                nc.scalar.activation(vrow[:, vsl], vpre[:], AF.Tanh)
                yield
                # V: kbc broadcast copy -> score pass for this level is go
                nc.vector.tensor_copy(kbc[:, ksl], pkb[:])

            def kt_load(l, ch):
                kt = ktp.tile([128, SUB, DK], f32, name="kt", bufs=3)
                nc.scalar.dma_start(
                    kt[:],
                    Kp[l].rearrange("(p t) d -> p t d", t=T)
                    [:, ch * SUB:(ch + 1) * SUB, :])
                return kt

            for _ in mlp_gen(0):
                pass
            kts = {(0, ch): kt_load(0, ch) for ch in range(3)}
            for l in range(L):
                ksl = slice(l * DK, (l + 1) * DK)
                kr = kres[l]
                nxt = iter(mlp_gen(l + 1)) if l + 1 < L else None
                if nxt is not None:
                    next(nxt, None)
                for ch in range(NCH):
                    kt = kts.pop((l, ch))
                    for t in range(SUB):
                        j = ch * SUB + t
                        lt = l * T + j
                        jk = junkp.tile([128, 128], bf16, name="jk", bufs=2)
                        nc.vector.scalar_tensor_tensor(
                            out=jk[:], in0=kt[:, t, :], scalar=1.0,
                            in1=kbc[:, ksl], op0=A.mult, op1=A.mult,
                            accum_out=scores[:, lt:lt + 1])
                        if nxt is not None and j % 6 == 5:
                            next(nxt, None)
                    nc.vector.tensor_copy(
                        kr[:, ch * SUB:(ch + 1) * SUB, :], kt[:])
                    # prefetch 3 chunks ahead, emitted AFTER this chunk's
                    # reads so the bufs=3 slot-reuse WAR sees them
                    if ch + 3 < NCH:
                        kts[(l, ch + 3)] = kt_load(l, ch + 3)
                    elif l + 1 < L:
                        kts[(l + 1, ch + 3 - NCH)] = kt_load(l + 1,
                                                            ch + 3 - NCH)
                if nxt is not None:
                    for _ in nxt:
                        pass
                nc.scalar.activation(scores[:, l * T:(l + 1) * T],
                                     scores[:, l * T:(l + 1) * T], AF.Exp)
                nc.vector.tensor_reduce(zpart[:, l:l + 1],
                                        scores[:, l * T:(l + 1) * T],
                                        axis=AX.X, op=A.add)

                # normalizer partial-sum -> AR trigger first (critical
                # path); the exp-row transpose/bounce only feeds the ei
                # loads (~needed 20us later), so it goes after.
                pz1 = pmisc.tile([1, 1], f32, name="pz1", tag="pmisc")
                nc.tensor.matmul(pz1[:], lhsT=ones_col[:],
                                 rhs=zpart[:, l:l + 1], start=True, stop=True)
                z1 = zrp.tile([1, 8], f32, name="z1")
                nc.gpsimd.memset(z1[:], 0.0)
                nc.vector.tensor_copy(z1[:, 0:1], pz1[:])

                nc.gpsimd.dma_start(cc_ins[l][:], z1[:])
                if not SKIP_CC:
                    nc.gpsimd.collective_compute(
                        "AllReduce", A.add,
                        replica_groups=[list(range(NCORES))],
                        ins=[cc_ins[l].opt()], outs=[cc_outs[l].opt()])
                else:
                    nc.gpsimd.dma_start(cc_outs[l][:], cc_ins[l][:])

                pt = pmisc.tile([64, 128], f32, name="pt", tag="pmisc")
                nc.tensor.transpose(pt[:], scores[:, l * T:(l + 1) * T],
                                    ident[:])
                et = zrp.tile([64, 128], bf16, name="et", bufs=1)
                nc.vector.tensor_copy(et[:], pt[:])
                nc.scalar.dma_start(
                    ecr[l * S:(l + 1) * S].rearrange("(t s) -> t s", s=128),
                    et[:])

            # ---- AR consumers + phase 2, interleaved per level so that no
            # AR wait sits ahead of another level's work in any in-order
            # engine queue. Per level: prefetch half the ei (exp-row)
            # tiles, then zg (the AR wait), then inv/vq/kq, remaining ei,
            # then the streaming loop.
            inv = sm.tile([1, L], f32, name="inv")

            # ei/zg loads go on the scalar (HWDGE) queue, NOT gpsimd: the
            # collective_compute triggers occupy the gpsimd queue until
            # their AR completes, so anything queued behind them would
            # inherit the full AR latency.
            def ei_load(l, c):
                ei = eip.tile([1, SUB * 128], bf16, name="ei", bufs=4)
                nc.scalar.dma_start(
                    ei[:],
                    ecr[l * S + c * SUB * 128:
                        l * S + (c + 1) * SUB * 128].rearrange(
                        "(a x) -> a x", a=1))
                return ei

            # AR-consumer block for level l: ei prefetch, zg (the AR wait),
            # then inv/vq/kq. Consumer(0) runs before the first chunk loop;
            # consumer(l+1) is hoisted into the MIDDLE of level l's chunk
            # loop (after chunk 6's compute) so that by the time level l+1
            # streaming starts, vq/kq are already computed and the level
            # transition costs ~2 engine hops instead of ~8 (~25us each).
            eis_all = {}

            def ar_consumer(l):
                vsl = slice(l * DLVL, (l + 1) * DLVL)
                ksl = slice(l * DK, (l + 1) * DK)
                eis_all[l] = {c: ei_load(l, c) for c in range(3)}
                zg = zrp.tile([1, 8], f32, name="zg")
                nc.scalar.dma_start(zg[:], cc_outs[l][:])
                zrcp = zrp.tile([1, 1], f32, name="zrcp")
                nc.vector.reciprocal(zrcp[:], zg[:, 0:1])
                nc.vector.tensor_tensor(inv[:, l:l + 1], geff[:, l:l + 1],
                                        zrcp[:], op=A.mult)
                nc.vector.tensor_scalar(vq[:, vsl], vrow[:, vsl],
                                        inv[:, l:l + 1], None, A.mult)
                nc.vector.tensor_scalar(kq[:, ksl], krow[:, ksl],
                                        inv[:, l:l + 1], None, A.mult)

            ar_consumer(0)
            for l in range(L):
                vsl = slice(l * DLVL, (l + 1) * DLVL)
                ksl = slice(l * DK, (l + 1) * DK)
                eis = eis_all[l]
                keep_sc = keep_bc[:, l:l + 1]
                kr = kres[l]
                outv = out[l].rearrange("(p t) d -> p t d", t=T)
                for c in range(NCH):
                    cs = slice(c * SUB, (c + 1) * SUB)
                    ei = eis.pop(c)
                    if c + 3 < NCH:
                        eis[c + 3] = ei_load(l, c + 3)
                    mi = mip.tile([128, SUB, DLVL], f32, name="mi", bufs=5)
                    nc.sync.dma_start(
                        mi[:],
                        Mp[l].rearrange("(p t) d -> p t d", t=T)[:, cs, :])
                    ko = ktp.tile([128, SUB, DK], f32, name="kt", bufs=3)
                    for t in range(SUB):
                        et_sl = ei[:, t * 128:(t + 1) * 128]
                        j = c * SUB + t
                        pm = pmp.tile([128, DLVL], f32, name="pm", tag="pm")
                        nc.tensor.matmul(pm[:], lhsT=et_sl,
                                         rhs=vq[:, vsl], start=True, stop=True)
                        nc.vector.scalar_tensor_tensor(
                            out=mi[:, t, :], in0=mi[:, t, :],
                            scalar=keep_sc, in1=pm[:], op0=A.mult, op1=A.add)
                        pkk = pkp.tile([128, DK], f32, name="pkk", tag="pk")
                        nc.tensor.matmul(pkk[:], lhsT=et_sl,
                                         rhs=kq[:, ksl], start=True, stop=True)
                        nc.vector.scalar_tensor_tensor(
                            out=ko[:, t, :], in0=kr[:, j, :],
                            scalar=keep_sc, in1=pkk[:], op0=A.mult, op1=A.add)
                    nc.scalar.dma_start(outv[:, cs, 0:DLVL], mi[:])
                    nc.scalar.dma_start(outv[:, cs, DLVL:DLVL + DK], ko[:])
                    if c == 6 and l + 1 < L:
                        ar_consumer(l + 1)

    nc.compile()
    return nc


def _prep_in_maps(inputs):
    f32 = np.float32
    s_t = np.asarray(inputs["s_t"], f32)
    e_t = np.asarray(inputs["e_t"], f32)
    lc = np.asarray(inputs["level_contexts"], f32)
    W_ev0 = np.asarray(inputs["W_ev0"], f32)
    W_ev = np.asarray(inputs["W_ev"], f32)
    b_ev = np.asarray(inputs["b_ev"], f32)
    ln_g = np.asarray(inputs["ln_g"], f32)
    ln_b = np.asarray(inputs["ln_b"], f32)
    W_gate = np.asarray(inputs["W_gate"], f32)
    b_gate = np.asarray(inputs["b_gate"], f32)
    W_val = np.asarray(inputs["W_val"], f32)
    b_val = np.asarray(inputs["b_val"], f32)
    W_key = np.asarray(inputs["W_key"], f32)
    b_key = np.asarray(inputs["b_key"], f32)
    M = np.asarray(inputs["M"], f32)
    K = np.asarray(inputs["K"], f32)
    decay = np.asarray(inputs["decay"], f32)

    # unified MLP input per level: level 0 uses [s, e, 0-pad], levels 1-2 use
    # [s, ctx, e]; weight matrices padded/stacked to match.
    xcat = np.zeros((L, 1792), f32)
    xcat[0, :1024] = s_t
    xcat[0, 1024:1536] = e_t
    for l in (1, 2):
        xcat[l] = np.concatenate([s_t, lc[l - 1], e_t])
    xcatT = np.ascontiguousarray(
        xcat.reshape(L, 14, 128).transpose(0, 2, 1))
    W0p = np.concatenate([W_ev0, np.zeros((DZ, 256), f32)], axis=1)
    Wfull = np.stack([W0p, W_ev[0], W_ev[1]])
    wevT = np.ascontiguousarray(Wfull.transpose(0, 2, 1))
    wvalT = np.ascontiguousarray(W_val.transpose(0, 2, 1))
    wkeyT = np.ascontiguousarray(W_key.transpose(0, 2, 1))

    shared = dict(
        xcatT=xcatT, wevT=wevT, wvalT=wvalT, wkeyT=wkeyT,
        bev_r=b_ev.reshape(1, -1), lng_r=ln_g.reshape(1, -1),
        lnb_r=ln_b.reshape(1, -1), wg_r=W_gate.reshape(1, -1),
        bg_r=b_gate.reshape(1, -1), bval_r=b_val.reshape(1, -1),
        bkey_r=b_key.reshape(1, -1), dec_r=decay.reshape(1, -1),
    )
    in_maps = []
    for c in range(NCORES):
        sl = slice(c * S, (c + 1) * S)
        m = dict(shared)
        m["Mp"] = np.ascontiguousarray(M[:, sl, :])
        m["Kp"] = np.ascontiguousarray(K[:, sl, :])
        in_maps.append(m)
    return in_maps


def _run(inputs, trace=False):
    import concourse.bass_utils as bass_utils

    nc = _STATE.get("nc")
    if nc is None:
        nc = _build_bass()
        _STATE["nc"] = nc
    in_maps = _prep_in_maps(inputs)
    res = bass_utils.run_bass_kernel_spmd(
        nc, in_maps, core_ids=list(range(NCORES)), trace=trace)
    full = np.concatenate([res.results[c]["out"] for c in range(NCORES)],
                          axis=1)
    return full.astype(np.float32, copy=False), res


def kernel(**inputs):
    out, _ = _run(inputs, trace=False)
    return out


# revision 5
# speedup vs baseline: 1.0263x; 1.0169x over previous
"""Trainium2 Bass kernel for nn_CMSWrite (scatter_memory) — final.

~463us HW exec (baseline: 531us), rel_l2 ~7e-4 (harness gate 2e-2).

Design (memory-bound problem; per-core traffic floor = read M+K 62.9MB +
write out 62.9MB = 125.8MB @ ~358GB/s = 351us):

- Slots (N=65536) sharded across 8 cores, 8192 each; tiny MLP replicated.
  Per level, partition p / tile-column j in [0,64) holds slot p*64+j; the
  same permutation is used by the score pass, exp-row bounce, M streaming
  and output writes, so it cancels.
- K is read ONCE per level: streamed as f32 chunks on the scalar (HWDGE)
  queue, scored from the f32 staging tile, and DVE-cast into a bf16
  resident SBUF tile (48KB/partition for all 3 levels) that later serves
  the K_new update. Phase 2 then streams only M (duplicate K read
  eliminated; bf16 rounding of keep*K costs ~7e-4 rel_l2).
- The 3 per-level softmax-normalizer AllReduces (32B each, 26-30us cold,
  ~10us warm, first-collective barrier ~50us after start) are fully
  pipelined: level l+1's MLP chain is software-pipelined INTO level l's
  64-op score pass in engine-run groups (cross-engine hop latency ~3us
  dominates the LN chain), triggers fire back to back on the gpsimd queue
  (collective_compute triggers BLOCK that queue, so nothing else lives
  there), and each level's AR consumers (ei prefetch, zg, inv/vq/kq) are
  hoisted into the middle of the previous level's chunk loop so a level
  transition costs ~2 hops instead of ~8.
- Phase 2 per 1024-slot chunk: rank-1 PE matmuls (bf16 exp-row x vq/kq)
  into PSUM, one fused DVE op per tile for M (in-place in the streamed
  tile) and K (f32 staging reusing the kt pool slots), 16KB-contiguous
  M loads (bufs=5 prefetch), 2KB/512B-run output writes.
"""

import math
import numpy as np

L = 3
N = 65536
DLVL = 512
DK = 128
DZ = 128
NCORES = 8
S = N // NCORES          # 8192 slots per core
T = S // 128             # 64 slot-tiles of 128
SUB = 8                  # tile-columns per phase-2 chunk
NCH = T // SUB           # 8 chunks per level
EPS = 1e-5
THRESH = 0.1
SCALE = 1.0 / math.sqrt(DK)

_STATE = {}
SKIP_CC = False   # debug: replace AllReduce with local copy (wrong numerics)


def _build_bass():
    import concourse.bacc as bacc
    import concourse.tile as tile
    import concourse.mybir as mybir
    from concourse.masks import make_identity

    f32 = mybir.dt.float32
    bf16 = mybir.dt.bfloat16
    A = mybir.AluOpType
    AF = mybir.ActivationFunctionType
    AX = mybir.AxisListType

    nc = bacc.Bacc("TRN2", target_bir_lowering=False, debug=False,
                   num_devices=NCORES)

    Mp = nc.dram_tensor("Mp", [L, S, DLVL], f32, kind="ExternalInput").ap()
    Kp = nc.dram_tensor("Kp", [L, S, DK], f32, kind="ExternalInput").ap()
    xcatT = nc.dram_tensor("xcatT", [L, 128, 14], f32, kind="ExternalInput").ap()
    wevT = nc.dram_tensor("wevT", [L, 1792, 128], f32, kind="ExternalInput").ap()
    wvalT = nc.dram_tensor("wvalT", [L, 128, DLVL], f32, kind="ExternalInput").ap()
    wkeyT = nc.dram_tensor("wkeyT", [L, 128, DK], f32, kind="ExternalInput").ap()
    bev_r = nc.dram_tensor("bev_r", [1, L * DZ], f32, kind="ExternalInput").ap()
    lng_r = nc.dram_tensor("lng_r", [1, L * DZ], f32, kind="ExternalInput").ap()
    lnb_r = nc.dram_tensor("lnb_r", [1, L * DZ], f32, kind="ExternalInput").ap()
    wg_r = nc.dram_tensor("wg_r", [1, L * DZ], f32, kind="ExternalInput").ap()
    bg_r = nc.dram_tensor("bg_r", [1, L], f32, kind="ExternalInput").ap()
    bval_r = nc.dram_tensor("bval_r", [1, L * DLVL], f32, kind="ExternalInput").ap()
    bkey_r = nc.dram_tensor("bkey_r", [1, L * DK], f32, kind="ExternalInput").ap()
    dec_r = nc.dram_tensor("dec_r", [1, L], f32, kind="ExternalInput").ap()

    out = nc.dram_tensor("out", [L, S, DLVL + DK], f32, kind="ExternalOutput").ap()

    with tile.TileContext(nc) as tc:
        with (
            tc.tile_pool(name="constp", bufs=1) as constp,
            tc.tile_pool(name="wp", bufs=1) as wp,
            tc.tile_pool(name="sm", bufs=1) as sm,
            tc.tile_pool(name="zrp", bufs=3) as zrp,
            tc.tile_pool(name="junkp", bufs=3) as junkp,
            tc.tile_pool(name="kresp", bufs=1) as kresp,
            tc.tile_pool(name="ktp", bufs=1) as ktp,
            tc.tile_pool(name="mip", bufs=1) as mip,
            tc.tile_pool(name="eip", bufs=1) as eip,
            tc.tile_pool(name="pmisc", bufs=3, space="PSUM") as pmisc,
            tc.tile_pool(name="pmp", bufs=3, space="PSUM") as pmp,
            tc.tile_pool(name="pkp", bufs=2, space="PSUM") as pkp,
            tc.tile_pool(name="dramp", bufs=1, space="DRAM") as dramp,
        ):
            # ---------------- constants / small input rows ----------------
            ident = constp.tile([128, 128], f32, name="ident")
            make_identity(nc, ident[:])
            ones_row = constp.tile([1, 128], f32, name="ones_row")
            nc.gpsimd.memset(ones_row[:], 1.0)
            ones_col = constp.tile([128, 1], f32, name="ones_col")
            nc.gpsimd.memset(ones_col[:], 1.0)
            eps_sb = constp.tile([1, 1], f32, name="eps_sb")
            nc.gpsimd.memset(eps_sb[:], EPS)

            def _row(name, src, width):
                t = sm.tile([1, width], f32, name=name)
                nc.sync.dma_start(t[:], src)
                return t

            bev_sb = _row("bev_sb", bev_r[:], L * DZ)
            lng_sb = _row("lng_sb", lng_r[:], L * DZ)
            lnb_sb = _row("lnb_sb", lnb_r[:], L * DZ)
            wg_sb = _row("wg_sb", wg_r[:], L * DZ)
            bg_sb = _row("bg_sb", bg_r[:], L)
            bval_sb = _row("bval_sb", bval_r[:], L * DLVL)
            bkey_sb = _row("bkey_sb", bkey_r[:], L * DK)
            dec_sb = _row("dec_sb", dec_r[:], L)

            # keep = 1 - decay, broadcast to all 128 partitions
            keepr = sm.tile([1, L], f32, name="keepr")
            nc.scalar.activation(keepr[:], dec_sb[:], AF.Identity,
                                 bias=1.0, scale=-1.0)
            pkeep = pmisc.tile([128, L], f32, name="pkeep", tag="pmisc")
            nc.tensor.matmul(pkeep[:], lhsT=ones_row[:], rhs=keepr[:],
                             start=True, stop=True)
            keep_bc = sm.tile([128, L], f32, name="keep_bc")
            nc.vector.tensor_copy(keep_bc[:], pkeep[:])

            # persistent per-level results (all on partition 0 rows)
            vrow = sm.tile([1, L * DLVL], bf16, name="vrow")
            krow = sm.tile([1, L * DK], f32, name="krow")
            # bf16 copies feed the phase-2 rank-1 matmuls (4x PE rate);
            # the update term is ~1e-4 of the output so bf16 noise ~1e-7.
            vq = sm.tile([1, L * DLVL], bf16, name="vq")
            kq = sm.tile([1, L * DK], bf16, name="kq")
            geff = sm.tile([1, L], f32, name="geff")
            kbc = sm.tile([128, L * DK], f32, name="kbc")
            scores = sm.tile([128, L * T], f32, name="scores")
            zpart = sm.tile([128, L], f32, name="zpart")

            ecr = dramp.tile([L * S], bf16, name="ecr")
            cc_ins = [dramp.tile([1, 8], f32, name=f"cc_in{l}")
                      for l in range(L)]
            cc_outs = [dramp.tile([1, 8], f32, name=f"cc_out{l}",
                                  addr_space="Shared") for l in range(L)]

            # ---------------- phase 0+1: per-level MLP chains + score
            # passes, software-pipelined. Level 0's MLP runs first (its
            # AR trigger is the critical one); level l+1's chain is then
            # interleaved into level l's 64-op score pass in engine-run
            # groups, so its ~9 cross-engine hops (~2-3us each) hide
            # inside the score window instead of serializing after it.
            # NO AR-consuming op is emitted here, so the 3 ARs pipeline.
            zS = [slice(l * DZ, (l + 1) * DZ) for l in range(L)]
            kres = []

            def mlp_gen(l):
                zsl = zS[l]
                ksl = slice(l * DK, (l + 1) * DK)
                vsl = slice(l * DLVL, (l + 1) * DLVL)
                # group 0: input loads (incl. this level's resident K) and
                # the z-preactivation PSUM chain on the PE.
                xc = wp.tile([128, 14], f32, name="xc", bufs=2)
                nc.sync.dma_start(xc[:], xcatT[l])
                wev = wp.tile([128, 14, 128], f32, name="wev", bufs=1)
                nc.sync.dma_start(
                    wev[:], wevT[l].rearrange("(c p) j -> p c j", p=128))
                kr = kresp.tile([128, T, DK], bf16, name=f"kres{l}")
                kres.append(kr)
                pz = pmisc.tile([1, 128], f32, name="pz", tag="pmisc")
                for c in range(14):
                    nc.tensor.matmul(pz[:], lhsT=xc[:, c:c + 1],
                                     rhs=wev[:, c, :],
                                     start=(c == 0), stop=(c == 13))
                yield
                # V-run: bias add + LN stats
                zr = zrp.tile([1, 128], f32, name="zr", bufs=2)
                nc.vector.tensor_tensor(zr[:], pz[:], bev_sb[:, zsl], op=A.add)
                musum = zrp.tile([1, 1], f32, name="musum", bufs=2)
                nc.vector.tensor_reduce(musum[:], zr[:], axis=AX.X, op=A.add)
                mu = zrp.tile([1, 1], f32, name="mu", bufs=2)
                nc.vector.tensor_scalar(mu[:], musum[:], 1.0 / DZ, None,
                                        A.mult)
                zm = zrp.tile([1, 128], f32, name="zm", bufs=2)
                nc.vector.tensor_scalar(zm[:], zr[:], mu[:], None, A.subtract)
                jr = junkp.tile([1, 128], f32, name="jr", bufs=2)
                nc.vector.tensor_tensor(jr[:], zm[:], zm[:], op=A.mult)
                vsum = zrp.tile([1, 1], f32, name="vsum", bufs=2)
                nc.vector.tensor_reduce(vsum[:], jr[:], axis=AX.X, op=A.add)
                yield
                # ACT hop: std; also drop the wval/wkey loads here so they
                # sit late in the sync queue (behind the K loads).
                wval = wp.tile([128, DLVL], f32, name="wval", bufs=2)
                nc.sync.dma_start(wval[:], wvalT[l])
                wkey = wp.tile([128, DK], f32, name="wkey", bufs=2)
                nc.sync.dma_start(wkey[:], wkeyT[l])
                std = zrp.tile([1, 1], f32, name="std", bufs=2)
                nc.scalar.activation(std[:], vsum[:], AF.Sqrt, bias=eps_sb[:],
                                     scale=1.0 / DZ)
                yield
                # V-run: normalize + relu + gate dot
                rstd = zrp.tile([1, 1], f32, name="rstd", bufs=2)
                nc.vector.reciprocal(rstd[:], std[:])
                zs2 = zrp.tile([1, 128], f32, name="zs2", bufs=2)
                nc.vector.scalar_tensor_tensor(
                    out=zs2[:], in0=zm[:], scalar=rstd[:],
                    in1=lng_sb[:, zsl], op0=A.mult, op1=A.mult)
                zs3 = zrp.tile([1, 128], f32, name="zs3", bufs=2)
                nc.vector.tensor_tensor(zs3[:], zs2[:], lnb_sb[:, zsl],
                                        op=A.add)
                zrow = zrp.tile([1, 128], f32, name="zrow", bufs=2)
                nc.vector.tensor_scalar(zrow[:], zs3[:], 0.0, None, A.max)
                jg = junkp.tile([1, 128], f32, name="jg", bufs=2)
                nc.vector.tensor_tensor(jg[:], zrow[:], wg_sb[:, zsl],
                                        op=A.mult)
                gd = zrp.tile([1, 1], f32, name="gd", bufs=2)
                nc.vector.tensor_reduce(gd[:], jg[:], axis=AX.X, op=A.add)
                yield
                # ACT hop: gate sigmoid; PE: z transpose
                gsig = zrp.tile([1, 1], f32, name="gsig", bufs=2)
                nc.scalar.activation(gsig[:], gd[:], AF.Sigmoid,
                                     bias=bg_sb[:, l:l + 1], scale=1.0)
                pzc = pmisc.tile([128, 1], f32, name="pzc", tag="pmisc")
                nc.tensor.transpose(pzc[:], zrow[:], ident[0:1, 0:1])
                yield
                # V-run: gate threshold + zcol copy
                msk = zrp.tile([1, 1], f32, name="msk", bufs=2)
                nc.vector.tensor_scalar(msk[:], gsig[:], THRESH, None,
                                        A.is_ge)
                nc.vector.tensor_tensor(geff[:, l:l + 1], gsig[:], msk[:],
                                        op=A.mult)
                zcol = zrp.tile([128, 1], f32, name="zcol", bufs=2)
                nc.vector.tensor_copy(zcol[:], pzc[:])
                yield
                # PE: value/key matvecs
                pv = pmisc.tile([1, DLVL], f32, name="pv", tag="pmisc")
                nc.tensor.matmul(pv[:], lhsT=zcol[:], rhs=wval[:],
                                 start=True, stop=True)
                pk0 = pmisc.tile([1, DK], f32, name="pk0", tag="pmisc")
                nc.tensor.matmul(pk0[:], lhsT=zcol[:], rhs=wkey[:],
                                 start=True, stop=True)
                yield
                # V-run: krow / ksc / vpre
                nc.vector.tensor_tensor(krow[:, ksl], pk0[:], bkey_sb[:, ksl],
                                        op=A.add)
                ksc = zrp.tile([1, DK], f32, name="ksc", bufs=2)
                nc.vector.tensor_scalar(ksc[:], krow[:, ksl], SCALE, None,
                                        A.mult)
                vpre = zrp.tile([1, DLVL], f32, name="vpre", bufs=2)
                nc.vector.tensor_tensor(vpre[:], pv[:], bval_sb[:, vsl],
                                        op=A.add)
                yield
                # PE: k broadcast; ACT: tanh (off the score critical path)
                pkb = pmisc.tile([128, DK], f32, name="pkb", tag="pmisc")
                nc.tensor.matmul(pkb[:], lhsT=ones_row[:], rhs=ksc[:],
                                 start=True, stop=True)
                nc.scalar.activation(vrow[:, vsl], vpre[:], AF.Tanh)
                yield
                # V: kbc broadcast copy -> score pass for this level is go
                nc.vector.tensor_copy(kbc[:, ksl], pkb[:])

            def kt_load(l, ch):
                kt = ktp.tile([128, SUB, DK], f32, name="kt", bufs=3)
                nc.scalar.dma_start(
                    kt[:],
                    Kp[l].rearrange("(p t) d -> p t d", t=T)
                    [:, ch * SUB:(ch + 1) * SUB, :])
                return kt

            for _ in mlp_gen(0):
                pass
            kts = {(0, ch): kt_load(0, ch) for ch in range(3)}
            for l in range(L):
                ksl = slice(l * DK, (l + 1) * DK)
                kr = kres[l]
                nxt = iter(mlp_gen(l + 1)) if l + 1 < L else None
                if nxt is not None:
                    next(nxt, None)
                for ch in range(NCH):
                    kt = kts.pop((l, ch))
                    for t in range(SUB):
                        j = ch * SUB + t
                        lt = l * T + j
                        jk = junkp.tile([128, 128], bf16, name="jk", bufs=2)
                        nc.vector.scalar_tensor_tensor(
                            out=jk[:], in0=kt[:, t, :], scalar=1.0,
                            in1=kbc[:, ksl], op0=A.mult, op1=A.mult,
                            accum_out=scores[:, lt:lt + 1])
                        if nxt is not None and j % 6 == 5:
                            next(nxt, None)
                    nc.vector.tensor_copy(
                        kr[:, ch * SUB:(ch + 1) * SUB, :], kt[:])
                    # prefetch 3 chunks ahead, emitted AFTER this chunk's
                    # reads so the bufs=3 slot-reuse WAR sees them
                    if ch + 3 < NCH:
                        kts[(l, ch + 3)] = kt_load(l, ch + 3)
                    elif l + 1 < L:
                        kts[(l + 1, ch + 3 - NCH)] = kt_load(l + 1,
                                                            ch + 3 - NCH)
                if nxt is not None:
                    for _ in nxt:
                        pass
                nc.scalar.activation(scores[:, l * T:(l + 1) * T],
                                     scores[:, l * T:(l + 1) * T], AF.Exp)
                nc.vector.tensor_reduce(zpart[:, l:l + 1],
                                        scores[:, l * T:(l + 1) * T],
                                        axis=AX.X, op=A.add)

                # normalizer partial-sum -> AR trigger first (critical
                # path); the exp-row transpose/bounce only feeds the ei
                # loads (~needed 20us later), so it goes after.
                pz1 = pmisc.tile([1, 1], f32, name="pz1", tag="pmisc")
                nc.tensor.matmul(pz1[:], lhsT=ones_col[:],
                                 rhs=zpart[:, l:l + 1], start=True, stop=True)
                z1 = zrp.tile([1, 8], f32, name="z1")
                nc.gpsimd.memset(z1[:], 0.0)
                nc.vector.tensor_copy(z1[:, 0:1], pz1[:])

                nc.gpsimd.dma_start(cc_ins[l][:], z1[:])
                if not SKIP_CC:
                    nc.gpsimd.collective_compute(
                        "AllReduce", A.add,
                        replica_groups=[list(range(NCORES))],
                        ins=[cc_ins[l].opt()], outs=[cc_outs[l].opt()])
                else:
                    nc.gpsimd.dma_start(cc_outs[l][:], cc_ins[l][:])

                pt = pmisc.tile([64, 128], f32, name="pt", tag="pmisc")
                nc.tensor.transpose(pt[:], scores[:, l * T:(l + 1) * T],
                                    ident[:])
                et = zrp.tile([64, 128], bf16, name="et", bufs=1)
                nc.vector.tensor_copy(et[:], pt[:])
                nc.scalar.dma_start(
                    ecr[l * S:(l + 1) * S].rearrange("(t s) -> t s", s=128),
                    et[:])

            # ---- AR consumers + phase 2, interleaved per level so that no
            # AR wait sits ahead of another level's work in any in-order
            # engine queue. Per level: prefetch half the ei (exp-row)
            # tiles, then zg (the AR wait), then inv/vq/kq, remaining ei,
            # then the streaming loop.
            inv = sm.tile([1, L], f32, name="inv")

            # ei/zg loads go on the scalar (HWDGE) queue, NOT gpsimd: the
            # collective_compute triggers occupy the gpsimd queue until
            # their AR completes, so anything queued behind them would
            # inherit the full AR latency.
            def ei_load(l, c):
                ei = eip.tile([1, SUB * 128], bf16, name="ei", bufs=4)
                nc.scalar.dma_start(
                    ei[:],
                    ecr[l * S + c * SUB * 128:
                        l * S + (c + 1) * SUB * 128].rearrange(
                        "(a x) -> a x", a=1))
                return ei

            # AR-consumer block for level l: ei prefetch, zg (the AR wait),
            # then inv/vq/kq. Consumer(0) runs before the first chunk loop;
            # consumer(l+1) is hoisted into the MIDDLE of level l's chunk
            # loop (after chunk 6's compute) so that by the time level l+1
            # streaming starts, vq/kq are already computed and the level
            # transition costs ~2 engine hops instead of ~8 (~25us each).
            eis_all = {}

            def ar_consumer(l):
                vsl = slice(l * DLVL, (l + 1) * DLVL)
                ksl = slice(l * DK, (l + 1) * DK)
                eis_all[l] = {c: ei_load(l, c) for c in range(3)}
                zg = zrp.tile([1, 8], f32, name="zg")
                nc.scalar.dma_start(zg[:], cc_outs[l][:])
                zrcp = zrp.tile([1, 1], f32, name="zrcp")
                nc.vector.reciprocal(zrcp[:], zg[:, 0:1])
                nc.vector.tensor_tensor(inv[:, l:l + 1], geff[:, l:l + 1],
                                        zrcp[:], op=A.mult)
                nc.vector.tensor_scalar(vq[:, vsl], vrow[:, vsl],
                                        inv[:, l:l + 1], None, A.mult)
                nc.vector.tensor_scalar(kq[:, ksl], krow[:, ksl],
                                        inv[:, l:l + 1], None, A.mult)

            ar_consumer(0)
            for l in range(L):
                vsl = slice(l * DLVL, (l + 1) * DLVL)
                ksl = slice(l * DK, (l + 1) * DK)
                eis = eis_all[l]
                keep_sc = keep_bc[:, l:l + 1]
                kr = kres[l]
                outv = out[l].rearrange("(p t) d -> p t d", t=T)
                for c in range(NCH):
                    cs = slice(c * SUB, (c + 1) * SUB)
                    ei = eis.pop(c)
                    if c + 3 < NCH:
                        eis[c + 3] = ei_load(l, c + 3)
                    mi = mip.tile([128, SUB, DLVL], f32, name="mi", bufs=5)
                    nc.sync.dma_start(
                        mi[:],
                        Mp[l].rearrange("(p t) d -> p t d", t=T)[:, cs, :])
                    ko = ktp.tile([128, SUB, DK], f32, name="kt", bufs=3)
                    for t in range(SUB):
                        et_sl = ei[:, t * 128:(t + 1) * 128]
                        j = c * SUB + t
                        pm = pmp.tile([128, DLVL], f32, name="pm", tag="pm")
                        nc.tensor.matmul(pm[:], lhsT=et_sl,
                                         rhs=vq[:, vsl], start=True, stop=True)
                        nc.vector.scalar_tensor_tensor(
                            out=mi[:, t, :], in0=mi[:, t, :],
                            scalar=keep_sc, in1=pm[:], op0=A.mult, op1=A.add)
                        pkk = pkp.tile([128, DK], f32, name="pkk", tag="pk")
                        nc.tensor.matmul(pkk[:], lhsT=et_sl,
                                         rhs=kq[:, ksl], start=True, stop=True)
                        nc.vector.scalar_tensor_tensor(
                            out=ko[:, t, :], in0=kr[:, j, :],
                            scalar=keep_sc, in1=pkk[:], op0=A.mult, op1=A.add)
                    nc.scalar.dma_start(outv[:, cs, 0:DLVL], mi[:])
                    nc.scalar.dma_start(outv[:, cs, DLVL:DLVL + DK], ko[:])
                    if c == 6 and l + 1 < L:
                        ar_consumer(l + 1)

    nc.compile()
    return nc


def _prep_in_maps(inputs):
    f32 = np.float32
    s_t = np.asarray(inputs["s_t"], f32)
    e_t = np.asarray(inputs["e_t"], f32)
    lc = np.asarray(inputs["level_contexts"], f32)
    W_ev0 = np.asarray(inputs["W_ev0"], f32)
    W_ev = np.asarray(inputs["W_ev"], f32)
    b_ev = np.asarray(inputs["b_ev"], f32)
    ln_g = np.asarray(inputs["ln_g"], f32)
    ln_b = np.asarray(inputs["ln_b"], f32)
    W_gate = np.asarray(inputs["W_gate"], f32)
    b_gate = np.asarray(inputs["b_gate"], f32)
    W_val = np.asarray(inputs["W_val"], f32)
    b_val = np.asarray(inputs["b_val"], f32)
    W_key = np.asarray(inputs["W_key"], f32)
    b_key = np.asarray(inputs["b_key"], f32)
    M = np.asarray(inputs["M"], f32)
    K = np.asarray(inputs["K"], f32)
    decay = np.asarray(inputs["decay"], f32)

    # unified MLP input per level: level 0 uses [s, e, 0-pad], levels 1-2 use
    # [s, ctx, e]; weight matrices padded/stacked to match.
    xcat = np.zeros((L, 1792), f32)
    xcat[0, :1024] = s_t
    xcat[0, 1024:1536] = e_t
    for l in (1, 2):
        xcat[l] = np.concatenate([s_t, lc[l - 1], e_t])
    xcatT = np.ascontiguousarray(
        xcat.reshape(L, 14, 128).transpose(0, 2, 1))
    W0p = np.concatenate([W_ev0, np.zeros((DZ, 256), f32)], axis=1)
    Wfull = np.stack([W0p, W_ev[0], W_ev[1]])
    wevT = np.ascontiguousarray(Wfull.transpose(0, 2, 1))
    wvalT = np.ascontiguousarray(W_val.transpose(0, 2, 1))
    wkeyT = np.ascontiguousarray(W_key.transpose(0, 2, 1))

    shared = dict(
        xcatT=xcatT, wevT=wevT, wvalT=wvalT, wkeyT=wkeyT,
        bev_r=b_ev.reshape(1, -1), lng_r=ln_g.reshape(1, -1),
        lnb_r=ln_b.reshape(1, -1), wg_r=W_gate.reshape(1, -1),
        bg_r=b_gate.reshape(1, -1), bval_r=b_val.reshape(1, -1),
        bkey_r=b_key.reshape(1, -1), dec_r=decay.reshape(1, -1),
    )
    in_maps = []
    for c in range(NCORES):
        sl = slice(c * S, (c + 1) * S)
        m = dict(shared)
        m["Mp"] = np.ascontiguousarray(M[:, sl, :])
        m["Kp"] = np.ascontiguousarray(K[:, sl, :])
        in_maps.append(m)
    return in_maps


def _run(inputs, trace=False):
    import concourse.bass_utils as bass_utils

    nc = _STATE.get("nc")
    if nc is None:
        nc = _build_bass()
        _STATE["nc"] = nc
    in_maps = _prep_in_maps(inputs)
    res = bass_utils.run_bass_kernel_spmd(
        nc, in_maps, core_ids=list(range(NCORES)), trace=trace)
    full = np.concatenate([res.results[c]["out"] for c in range(NCORES)],
                          axis=1)
    return full.astype(np.float32, copy=False), res


def kernel(**inputs):
    out, _ = _run(inputs, trace=False)
    return out
